# revision 18
# baseline (speedup 1.0000x reference)
"""Trainium2 Bass kernel for nn_ADJ_AttCenter (gnn_message_passing).

Reference computation (N=8192, C=2048):
  w = x @ att_w.T + att_b                  # [N,1] attention scores
  w normalized per 2048-row segment (4 segments)
  centers = per-segment weighted sum of x  # [4, C]
  x_new = concat([x, centers])             # [N+4, C]
  adj   = degree-normalized constant adjacency (identity + center links)
                                           # [N+4, N+4]

Distribution over 8 NeuronCores: x and the adj rows are row-sharded
(1024 rows per core). Each core computes its local attention scores and
unnormalized center partials ([1, C+1] including the score sum), a
single small AllGather shares the 8 partials, and every core finishes
the center normalization locally. The adj row block per core is written
as structured constant spans directly from two constant SBUF tiles,
each chunk of 128 rows taking exactly two DMAs:
  - cols [0,(j+1)*128): zeros with the 0.5*I diagonal block as the last
    128 columns (sliding window into zid_t)
  - cols [(j+1)*128, 8196): zeros with the center-link value landing at
    column 8192 (sliding window into zc_t)
To keep the SPMD instruction stream identical across cores, each core
writes its adj block with columns rotated left by core*1024; the host
undoes the rotation with a np.roll per core when assembling.
"""

import numpy as np
import concourse.bass as bass
import concourse.bacc as bacc
import concourse.tile as tile
import concourse.mybir as mybir
from concourse import bass_utils

N = 8192
C = 2048
N_CORES = 8
ROWS = N // N_CORES          # 1024 rows of x / adj per core
SEG = N // 4                 # 2048 rows per attention segment
P = 128                      # SBUF partitions
NCHUNK = ROWS // P           # 8 chunks of 128 rows per core
W_ADJ = N + 4                # 8196 adj columns

# Degree-normalization constants, computed in float32 exactly as the
# reference does: d = rowsum ** -0.5, entries = (A*d).T*d.
_f32 = np.float32
_d_reg = _f32(_f32(2.0) ** _f32(-0.5))        # regular rows: deg 2
_d_c = _f32(_f32(SEG + 4) ** _f32(-0.5))      # center rows: deg 2052
C_DIAG = float(_f32(_d_reg * _d_reg))         # diagonal of regular rows
C_LINK = float(_f32(_d_reg * _d_c))           # segment-row <-> center link
C_CC = float(_f32(_d_c * _d_c))               # center 4x4 block

_CACHED_NC = None


def _build():
    nc = bacc.Bacc("TRN2", target_bir_lowering=False, debug=False,
                   num_devices=N_CORES)
    f32 = mybir.dt.float32

    x_in = nc.dram_tensor("x", [ROWS, C], f32, kind="ExternalInput")
    att_w = nc.dram_tensor("att_w", [1, C], f32, kind="ExternalInput")
    att_b = nc.dram_tensor("att_b", [1, 1], f32, kind="ExternalInput")

    out_x = nc.dram_tensor("out_x", [ROWS, C], f32, kind="ExternalOutput")
    out_centers = nc.dram_tensor("out_centers", [4, C], f32,
                                 kind="ExternalOutput")
    out_adj = nc.dram_tensor("out_adj", [ROWS, W_ADJ], f32,
                             kind="ExternalOutput")
    out_adjc = nc.dram_tensor("out_adjc", [4, W_ADJ], f32,
                              kind="ExternalOutput")

    with tile.TileContext(nc) as tc:
        with (
            tc.tile_pool(name="const", bufs=1) as const,
            tc.tile_pool(name="xp", bufs=NCHUNK) as xp,
            tc.tile_pool(name="scratch", bufs=1) as scratch,
            tc.tile_pool(name="wp", bufs=2) as wp,
            tc.tile_pool(name="small", bufs=1) as small,
            tc.tile_pool(name="psum", bufs=1, space="PSUM") as psum,
            tc.tile_pool(name="dram", bufs=1, space="DRAM") as dram,
        ):
            # ---- constant tiles ----
            # zc_t: zeros with C_LINK at column N-P. Chunk j's right span
            # (dest cols [(j+1)*128, 8196), width Wj = 8196-128*(j+1))
            # reads zc_t[:, 128*j : 128*j + Wj], landing C_LINK at dest
            # column 8192 for every j.
            ZC_WIDTH = W_ADJ - P   # 8068; max right-span width + slide room
            zc_t = const.tile([P, ZC_WIDTH], f32)
            nc.vector.memset(zc_t[:], 0.0)
            nc.vector.memset(zc_t[:, N - P:N - P + 1], C_LINK)

            # zid_t: zeros with the 0.5*I diagonal block as the last 128
            # cols. Chunk j's left span (dest cols [0, (j+1)*128)) reads
            # zid_t[:, (7-j)*128 : 1024].
            zid_t = const.tile([P, NCHUNK * P], f32)
            half_t = const.tile([P, P], f32)
            nc.gpsimd.memset(zid_t[:, 0:(NCHUNK - 1) * P], 0.0)
            nc.gpsimd.memset(half_t[:], C_DIAG)
            nc.gpsimd.affine_select(
                zid_t[:, (NCHUNK - 1) * P:NCHUNK * P], half_t[:],
                pattern=[[1, P]],
                compare_op=mybir.AluOpType.is_equal, fill=0.0,
                base=0, channel_multiplier=-1,
            )

            # center adj row sources (identical on every core, true layout)
            crow_t = const.tile([1, SEG], f32)
            nc.gpsimd.memset(crow_t[:], C_LINK)
            cc4_t = const.tile([4, 4], f32)
            nc.gpsimd.memset(cc4_t[:], C_CC)

            # attention weight/bias broadcast across partitions
            attw_stage = const.tile([1, C], f32)
            attw_rep = const.tile([P, C], f32)
            nc.sync.dma_start(attw_stage[:], att_w[:])
            nc.gpsimd.partition_broadcast(attw_rep[:], attw_stage[:])
            attb_stage = const.tile([1, 1], f32)
            attb_rep = const.tile([P, 1], f32)
            nc.sync.dma_start(attb_stage[:], att_b[:])
            nc.gpsimd.partition_broadcast(attb_rep[:], attb_stage[:])

            # ---- x stream: pass-through + attention partials ----
            # All 8 loads are emitted first and ride the sync (SP) HWDGE
            # ring ahead of everything else, so they drain at full HBM
            # rate; the adj bulk writes are queued on the same ring LAST,
            # giving the compute stream strict FIFO priority.
            ps = [psum.tile([1, 512], f32, name=f"ps{n}") for n in range(4)]
            ps_sum = psum.tile([1, 1], f32)

            xts = []
            for i in range(NCHUNK):
                rows = slice(i * P, (i + 1) * P)
                xt = xp.tile([P, C + 1], f32, name=f"xt{i}", tag="xt")
                nc.sync.dma_start(xt[:, 0:C], x_in[rows, :])
                nc.vector.memset(xt[:, C:C + 1], 1.0)
                xts.append(xt)

            for i in range(NCHUNK):
                rows = slice(i * P, (i + 1) * P)
                xt = xts[i]
                nc.scalar.dma_start(out_x[rows, :], xt[:, 0:C])

                # wcol = sum(x * att_w, axis=1) + att_b
                # (tensor_tensor_reduce would fuse this, but it faults the
                # device on TRN2 hardware — use mul + reduce + add)
                prod = scratch.tile([P, C], f32)
                wcol = wp.tile([P, 1], f32)
                wraw = wp.tile([P, 1], f32)
                nc.vector.tensor_mul(prod[:], xt[:, 0:C], attw_rep[:])
                nc.vector.tensor_reduce(wraw[:], prod[:],
                                        axis=mybir.AxisListType.X,
                                        op=mybir.AluOpType.add)
                nc.vector.tensor_scalar_add(wcol[:], wraw[:], attb_rep[:])
                # partials += wcol.T @ [x | 1]  (accumulated over chunks)
                for n in range(4):
                    nc.tensor.matmul(ps[n][:], wcol[:],
                                     xt[:, n * 512:(n + 1) * 512],
                                     start=(i == 0), stop=(i == NCHUNK - 1))
                nc.tensor.matmul(ps_sum[:], wcol[:], xt[:, C:C + 1],
                                 start=(i == 0), stop=(i == NCHUNK - 1))

            # ---- share partials, finish centers ----
            # pad the per-rank collective buffer to 2056 f32 (8224 B) so
            # each rank's block stays 32-byte aligned
            CP = C + 8
            partial = small.tile([1, CP], f32)
            nc.vector.memset(partial[:, C:CP], 0.0)
            for n in range(4):
                nc.vector.tensor_copy(partial[:, n * 512:(n + 1) * 512],
                                      ps[n][:])
            nc.vector.tensor_copy(partial[:, C:C + 1], ps_sum[:])

            agin = dram.tile([1, CP], f32)
            agout = dram.tile([N_CORES, CP], f32)
            nc.gpsimd.dma_start(agin[:], partial[:])
            nc.gpsimd.collective_compute(
                "AllGather", mybir.AluOpType.bypass,
                replica_groups=[list(range(N_CORES))],
                ins=[agin.opt()], outs=[agout.opt()],
            )
            # segment i partial = core 2i + core 2i+1
            # Tail DMAs ride gpsimd (SWDGE) so they don't queue behind the
            # adj bulk writes on the sync ring.
            ag3 = agout[:].rearrange("(a b) c -> a b c", b=2)
            ev = small.tile([4, C + 1], f32)
            od = small.tile([4, C + 1], f32)
            nc.gpsimd.dma_start(ev[:], ag3[:, 0, 0:C + 1])
            nc.gpsimd.dma_start(od[:], ag3[:, 1, 0:C + 1])
            sums = small.tile([4, C + 1], f32)
            nc.vector.tensor_add(sums[:], ev[:], od[:])
            recip = small.tile([4, 1], f32)
            nc.vector.reciprocal(recip[:], sums[:, C:C + 1])
            cent = small.tile([4, C], f32)
            nc.vector.tensor_scalar_mul(cent[:], sums[:, 0:C], recip[:])
            nc.gpsimd.dma_start(out_centers[:], cent[:])

            # center adj rows: zero spans + c_link segment + 4x4 cc block
            # (small; on the scalar ring so they run early and free)
            for i in range(4):
                row = slice(i, i + 1)
                if i > 0:
                    nc.scalar.dma_start(out_adjc[row, 0:i * SEG],
                                        zc_t[0:1, 0:i * SEG])
                nc.scalar.dma_start(out_adjc[row, i * SEG:(i + 1) * SEG],
                                    crow_t[:])
                if i < 3:
                    nc.scalar.dma_start(out_adjc[row, (i + 1) * SEG:N],
                                        zc_t[0:1, 0:N - (i + 1) * SEG])
            nc.scalar.dma_start(out_adjc[0:4, N:N + 4], cc4_t[:])

            # ---- adj row block: 2 DMAs per 128-row chunk ----
            # Emitted last on the sync ring: strict FIFO behind the x
            # stream, so they fill all remaining HBM bandwidth.
            for j in range(NCHUNK):
                rows = slice(j * P, (j + 1) * P)
                lw = (j + 1) * P                  # left span width
                nc.scalar.dma_start(out_adj[rows, 0:lw],
                                    zid_t[:, NCHUNK * P - lw:NCHUNK * P])
                rw = W_ADJ - lw                   # right span width
                nc.scalar.dma_start(out_adj[rows, lw:W_ADJ],
                                    zc_t[:, j * P:j * P + rw])

    nc.compile()
    return nc


def _get_nc():
    global _CACHED_NC
    if _CACHED_NC is None:
        _CACHED_NC = _build()
    return _CACHED_NC


def kernel(x, att_w, att_b):
    x = np.ascontiguousarray(np.asarray(x, dtype=np.float32))
    att_w = np.ascontiguousarray(np.asarray(att_w, dtype=np.float32))
    att_b = np.asarray(att_b, dtype=np.float32).reshape(1, 1)
    assert x.shape == (N, C) and att_w.shape == (1, C)

    nc = _get_nc()
    in_maps = [
        {
            "x": np.ascontiguousarray(x[k * ROWS:(k + 1) * ROWS]),
            "att_w": att_w,
            "att_b": att_b,
        }
        for k in range(N_CORES)
    ]
    try:
        res = bass_utils.run_bass_kernel_spmd(
            nc, in_maps, core_ids=list(range(N_CORES))
        )
    except Exception:
        # one retry to ride out transient runtime/worker hiccups
        res = bass_utils.run_bass_kernel_spmd(
            nc, in_maps, core_ids=list(range(N_CORES))
        )
    return assemble(res.results)


def assemble(results):
    x_new = np.empty((N + 4, C), dtype=np.float32)
    adj = np.empty((N + 4, W_ADJ), dtype=np.float32)
    for k in range(N_CORES):
        rows = slice(k * ROWS, (k + 1) * ROWS)
        x_new[rows] = results[k]["out_x"]
        blk = results[k]["out_adj"]
        # un-rotate the core-local column layout back to global columns
        adj[rows, 0:N] = np.roll(blk[:, 0:N], k * ROWS, axis=1)
        seg = k // 2
        adj[rows, N + seg] = blk[:, N]
        rest = [N + j for j in range(4) if j != seg]
        adj[rows, rest] = blk[:, N + 1:N + 4]
    x_new[N:] = results[0]["out_centers"]
    adj[N:] = results[0]["out_adjc"]
    return x_new, adj


# revision 19
# speedup vs baseline: 1.0435x; 1.0435x over previous
"""Trainium2 Bass kernel for nn_ADJ_AttCenter (gnn_message_passing).

Reference computation (N=8192, C=2048):
  w = x @ att_w.T + att_b                  # [N,1] attention scores
  w normalized per 2048-row segment (4 segments)
  centers = per-segment weighted sum of x  # [4, C]
  x_new = concat([x, centers])             # [N+4, C]
  adj   = degree-normalized constant adjacency (identity + center links)
                                           # [N+4, N+4]

Distribution over 8 NeuronCores: x and the adj rows are row-sharded
(1024 rows per core). Each core computes its local attention scores and
unnormalized center partials ([1, C+1] including the score sum), a
single small AllGather shares the 8 partials, and every core finishes
the center normalization locally. The adj row block per core is written
as structured constant spans directly from two constant SBUF tiles,
each chunk of 128 rows taking exactly two DMAs:
  - cols [0,(j+1)*128): zeros with the 0.5*I diagonal block as the last
    128 columns (sliding window into zid_t)
  - cols [(j+1)*128, 8196): zeros with the center-link value landing at
    column 8192 (sliding window into zc_t)
To keep the SPMD instruction stream identical across cores, each core
writes its adj block with columns rotated left by core*1024; the host
undoes the rotation with a np.roll per core when assembling.
"""

import numpy as np
import concourse.bass as bass
import concourse.bacc as bacc
import concourse.tile as tile
import concourse.mybir as mybir
from concourse import bass_utils

N = 8192
C = 2048
N_CORES = 8
ROWS = N // N_CORES          # 1024 rows of x / adj per core
SEG = N // 4                 # 2048 rows per attention segment
P = 128                      # SBUF partitions
NCHUNK = ROWS // P           # 8 chunks of 128 rows per core
W_ADJ = N + 4                # 8196 adj columns

# Degree-normalization constants, computed in float32 exactly as the
# reference does: d = rowsum ** -0.5, entries = (A*d).T*d.
_f32 = np.float32
_d_reg = _f32(_f32(2.0) ** _f32(-0.5))        # regular rows: deg 2
_d_c = _f32(_f32(SEG + 4) ** _f32(-0.5))      # center rows: deg 2052
C_DIAG = float(_f32(_d_reg * _d_reg))         # diagonal of regular rows
C_LINK = float(_f32(_d_reg * _d_c))           # segment-row <-> center link
C_CC = float(_f32(_d_c * _d_c))               # center 4x4 block

_CACHED_NC = None


def _build():
    nc = bacc.Bacc("TRN2", target_bir_lowering=False, debug=False,
                   num_devices=N_CORES)
    f32 = mybir.dt.float32

    x_in = nc.dram_tensor("x", [ROWS, C], f32, kind="ExternalInput")
    att_w = nc.dram_tensor("att_w", [1, C], f32, kind="ExternalInput")
    att_b = nc.dram_tensor("att_b", [1, 1], f32, kind="ExternalInput")

    out_x = nc.dram_tensor("out_x", [ROWS, C], f32, kind="ExternalOutput")
    out_centers = nc.dram_tensor("out_centers", [4, C], f32,
                                 kind="ExternalOutput")
    out_adj = nc.dram_tensor("out_adj", [ROWS, W_ADJ], f32,
                             kind="ExternalOutput")
    out_adjc = nc.dram_tensor("out_adjc", [4, W_ADJ], f32,
                              kind="ExternalOutput")

    with tile.TileContext(nc) as tc:
        with (
            tc.tile_pool(name="const", bufs=1) as const,
            tc.tile_pool(name="xp", bufs=NCHUNK) as xp,
            tc.tile_pool(name="scratch", bufs=1) as scratch,
            tc.tile_pool(name="wp", bufs=2) as wp,
            tc.tile_pool(name="small", bufs=1) as small,
            tc.tile_pool(name="psum", bufs=1, space="PSUM") as psum,
            tc.tile_pool(name="dram", bufs=1, space="DRAM") as dram,
        ):
            # ---- constant tiles ----
            # zc_t: zeros with C_LINK at column N-P. Chunk j's right span
            # (dest cols [(j+1)*128, 8196), width Wj = 8196-128*(j+1))
            # reads zc_t[:, 128*j : 128*j + Wj], landing C_LINK at dest
            # column 8192 for every j.
            ZC_WIDTH = W_ADJ - P   # 8068; max right-span width + slide room
            zc_t = const.tile([P, ZC_WIDTH], f32)
            nc.vector.memset(zc_t[:], 0.0)
            nc.vector.memset(zc_t[:, N - P:N - P + 1], C_LINK)

            # zid_t: zeros with the 0.5*I diagonal block as the last 128
            # cols. Chunk j's left span (dest cols [0, (j+1)*128)) reads
            # zid_t[:, (7-j)*128 : 1024].
            zid_t = const.tile([P, NCHUNK * P], f32)
            half_t = const.tile([P, P], f32)
            nc.gpsimd.memset(zid_t[:, 0:(NCHUNK - 1) * P], 0.0)
            nc.gpsimd.memset(half_t[:], C_DIAG)
            nc.gpsimd.affine_select(
                zid_t[:, (NCHUNK - 1) * P:NCHUNK * P], half_t[:],
                pattern=[[1, P]],
                compare_op=mybir.AluOpType.is_equal, fill=0.0,
                base=0, channel_multiplier=-1,
            )

            # center adj row sources (identical on every core, true layout)
            crow_t = const.tile([1, SEG], f32)
            nc.gpsimd.memset(crow_t[:], C_LINK)
            cc4_t = const.tile([4, 4], f32)
            nc.gpsimd.memset(cc4_t[:], C_CC)

            # attention weight/bias broadcast across partitions
            attw_stage = const.tile([1, C], f32)
            attw_rep = const.tile([P, C], f32)
            nc.sync.dma_start(attw_stage[:], att_w[:])
            nc.gpsimd.partition_broadcast(attw_rep[:], attw_stage[:])
            attb_stage = const.tile([1, 1], f32)
            attb_rep = const.tile([P, 1], f32)
            nc.sync.dma_start(attb_stage[:], att_b[:])
            nc.gpsimd.partition_broadcast(attb_rep[:], attb_stage[:])

            # ---- x stream: pass-through + attention partials ----
            # All 8 loads are emitted first and ride the sync (SP) HWDGE
            # ring ahead of everything else, so they drain at full HBM
            # rate; the adj bulk writes are queued on the same ring LAST,
            # giving the compute stream strict FIFO priority.
            ps = [psum.tile([1, 512], f32, name=f"ps{n}") for n in range(4)]
            ps_sum = psum.tile([1, 1], f32)

            xts = []
            for i in range(NCHUNK):
                rows = slice(i * P, (i + 1) * P)
                xt = xp.tile([P, C + 1], f32, name=f"xt{i}", tag="xt")
                nc.sync.dma_start(xt[:, 0:C], x_in[rows, :])
                nc.vector.memset(xt[:, C:C + 1], 1.0)
                xts.append(xt)

            for i in range(NCHUNK):
                rows = slice(i * P, (i + 1) * P)
                xt = xts[i]
                nc.sync.dma_start(out_x[rows, :], xt[:, 0:C])

                # wcol = sum(x * att_w, axis=1) + att_b
                # (tensor_tensor_reduce would fuse this, but it faults the
                # device on TRN2 hardware — use mul + reduce + add)
                prod = scratch.tile([P, C], f32)
                wcol = wp.tile([P, 1], f32)
                wraw = wp.tile([P, 1], f32)
                nc.vector.tensor_mul(prod[:], xt[:, 0:C], attw_rep[:])
                nc.vector.tensor_reduce(wraw[:], prod[:],
                                        axis=mybir.AxisListType.X,
                                        op=mybir.AluOpType.add)
                nc.vector.tensor_scalar_add(wcol[:], wraw[:], attb_rep[:])
                # partials += wcol.T @ [x | 1]  (accumulated over chunks)
                for n in range(4):
                    nc.tensor.matmul(ps[n][:], wcol[:],
                                     xt[:, n * 512:(n + 1) * 512],
                                     start=(i == 0), stop=(i == NCHUNK - 1))
                nc.tensor.matmul(ps_sum[:], wcol[:], xt[:, C:C + 1],
                                 start=(i == 0), stop=(i == NCHUNK - 1))

            # ---- share partials, finish centers ----
            # pad the per-rank collective buffer to 2056 f32 (8224 B) so
            # each rank's block stays 32-byte aligned
            CP = C + 8
            partial = small.tile([1, CP], f32)
            nc.vector.memset(partial[:, C:CP], 0.0)
            for n in range(4):
                nc.vector.tensor_copy(partial[:, n * 512:(n + 1) * 512],
                                      ps[n][:])
            nc.vector.tensor_copy(partial[:, C:C + 1], ps_sum[:])

            agin = dram.tile([1, CP], f32)
            agout = dram.tile([N_CORES, CP], f32)
            nc.gpsimd.dma_start(agin[:], partial[:])
            nc.gpsimd.collective_compute(
                "AllGather", mybir.AluOpType.bypass,
                replica_groups=[list(range(N_CORES))],
                ins=[agin.opt()], outs=[agout.opt()],
            )
            # segment i partial = core 2i + core 2i+1
            # Tail DMAs ride gpsimd (SWDGE) so they don't queue behind the
            # adj bulk writes on the sync ring.
            ag3 = agout[:].rearrange("(a b) c -> a b c", b=2)
            ev = small.tile([4, C + 1], f32)
            od = small.tile([4, C + 1], f32)
            nc.gpsimd.dma_start(ev[:], ag3[:, 0, 0:C + 1])
            nc.gpsimd.dma_start(od[:], ag3[:, 1, 0:C + 1])
            sums = small.tile([4, C + 1], f32)
            nc.vector.tensor_add(sums[:], ev[:], od[:])
            recip = small.tile([4, 1], f32)
            nc.vector.reciprocal(recip[:], sums[:, C:C + 1])
            cent = small.tile([4, C], f32)
            nc.vector.tensor_scalar_mul(cent[:], sums[:, 0:C], recip[:])
            nc.gpsimd.dma_start(out_centers[:], cent[:])

            # center adj rows: zero spans + c_link segment + 4x4 cc block
            # (small; on the scalar ring so they run early and free)
            for i in range(4):
                row = slice(i, i + 1)
                if i > 0:
                    nc.scalar.dma_start(out_adjc[row, 0:i * SEG],
                                        zc_t[0:1, 0:i * SEG])
                nc.scalar.dma_start(out_adjc[row, i * SEG:(i + 1) * SEG],
                                    crow_t[:])
                if i < 3:
                    nc.scalar.dma_start(out_adjc[row, (i + 1) * SEG:N],
                                        zc_t[0:1, 0:N - (i + 1) * SEG])
            nc.scalar.dma_start(out_adjc[0:4, N:N + 4], cc4_t[:])

            # ---- adj row block: 2 DMAs per 128-row chunk ----
            # Emitted last on the sync ring: strict FIFO behind the x
            # stream, so they fill all remaining HBM bandwidth.
            for j in range(NCHUNK):
                rows = slice(j * P, (j + 1) * P)
                lw = (j + 1) * P                  # left span width
                nc.sync.dma_start(out_adj[rows, 0:lw],
                                  zid_t[:, NCHUNK * P - lw:NCHUNK * P])
                rw = W_ADJ - lw                   # right span width
                nc.sync.dma_start(out_adj[rows, lw:W_ADJ],
                                  zc_t[:, j * P:j * P + rw])

    nc.compile()
    return nc


def _get_nc():
    global _CACHED_NC
    if _CACHED_NC is None:
        _CACHED_NC = _build()
    return _CACHED_NC


def kernel(x, att_w, att_b):
    x = np.ascontiguousarray(np.asarray(x, dtype=np.float32))
    att_w = np.ascontiguousarray(np.asarray(att_w, dtype=np.float32))
    att_b = np.asarray(att_b, dtype=np.float32).reshape(1, 1)
    assert x.shape == (N, C) and att_w.shape == (1, C)

    nc = _get_nc()
    in_maps = [
        {
            "x": np.ascontiguousarray(x[k * ROWS:(k + 1) * ROWS]),
            "att_w": att_w,
            "att_b": att_b,
        }
        for k in range(N_CORES)
    ]
    try:
        res = bass_utils.run_bass_kernel_spmd(
            nc, in_maps, core_ids=list(range(N_CORES))
        )
    except Exception:
        # one retry to ride out transient runtime/worker hiccups
        res = bass_utils.run_bass_kernel_spmd(
            nc, in_maps, core_ids=list(range(N_CORES))
        )
    return assemble(res.results)


def assemble(results):
    x_new = np.empty((N + 4, C), dtype=np.float32)
    adj = np.empty((N + 4, W_ADJ), dtype=np.float32)
    for k in range(N_CORES):
        rows = slice(k * ROWS, (k + 1) * ROWS)
        x_new[rows] = results[k]["out_x"]
        blk = results[k]["out_adj"]
        # un-rotate the core-local column layout back to global columns
        adj[rows, 0:N] = np.roll(blk[:, 0:N], k * ROWS, axis=1)
        seg = k // 2
        adj[rows, N + seg] = blk[:, N]
        rest = [N + j for j in range(4) if j != seg]
        adj[rows, rest] = blk[:, N + 1:N + 4]
    x_new[N:] = results[0]["out_centers"]
    adj[N:] = results[0]["out_adjc"]
    return x_new, adj


# revision 21
# speedup vs baseline: 1.0448x; 1.0013x over previous
"""Trainium2 Bass kernel for nn_ADJ_AttCenter (gnn_message_passing).

Reference computation (N=8192, C=2048):
  w = x @ att_w.T + att_b                  # [N,1] attention scores
  w normalized per 2048-row segment (4 segments)
  centers = per-segment weighted sum of x  # [4, C]
  x_new = concat([x, centers])             # [N+4, C]
  adj   = degree-normalized constant adjacency (identity + center links)
                                           # [N+4, N+4]

Distribution over 8 NeuronCores: x and the adj rows are row-sharded
(1024 rows per core). Each core computes its local attention scores and
unnormalized center partials ([1, C+1] including the score sum), a
single small AllGather shares the 8 partials, and every core finishes
the center normalization locally. The adj row block per core is written
as structured constant spans directly from two constant SBUF tiles,
each chunk of 128 rows taking exactly two DMAs:
  - cols [0,(j+1)*128): zeros with the 0.5*I diagonal block as the last
    128 columns (sliding window into zid_t)
  - cols [(j+1)*128, 8196): zeros with the center-link value landing at
    column 8192 (sliding window into zc_t)
To keep the SPMD instruction stream identical across cores, each core
writes its adj block with columns rotated left by core*1024; the host
undoes the rotation with a np.roll per core when assembling.
"""

import numpy as np
import concourse.bass as bass
import concourse.bacc as bacc
import concourse.tile as tile
import concourse.mybir as mybir
from concourse import bass_utils

N = 8192
C = 2048
N_CORES = 8
ROWS = N // N_CORES          # 1024 rows of x / adj per core
SEG = N // 4                 # 2048 rows per attention segment
P = 128                      # SBUF partitions
NCHUNK = ROWS // P           # 8 chunks of 128 rows per core
W_ADJ = N + 4                # 8196 adj columns

# Degree-normalization constants, computed in float32 exactly as the
# reference does: d = rowsum ** -0.5, entries = (A*d).T*d.
_f32 = np.float32
_d_reg = _f32(_f32(2.0) ** _f32(-0.5))        # regular rows: deg 2
_d_c = _f32(_f32(SEG + 4) ** _f32(-0.5))      # center rows: deg 2052
C_DIAG = float(_f32(_d_reg * _d_reg))         # diagonal of regular rows
C_LINK = float(_f32(_d_reg * _d_c))           # segment-row <-> center link
C_CC = float(_f32(_d_c * _d_c))               # center 4x4 block

_CACHED_NC = None


def _build():
    nc = bacc.Bacc("TRN2", target_bir_lowering=False, debug=False,
                   num_devices=N_CORES)
    f32 = mybir.dt.float32

    x_in = nc.dram_tensor("x", [ROWS, C], f32, kind="ExternalInput")
    att_w = nc.dram_tensor("att_w", [1, C], f32, kind="ExternalInput")
    att_b = nc.dram_tensor("att_b", [1, 1], f32, kind="ExternalInput")

    out_x = nc.dram_tensor("out_x", [ROWS, C], f32, kind="ExternalOutput")
    out_centers = nc.dram_tensor("out_centers", [4, C], f32,
                                 kind="ExternalOutput")
    out_adj = nc.dram_tensor("out_adj", [ROWS, W_ADJ], f32,
                             kind="ExternalOutput")
    out_adjc = nc.dram_tensor("out_adjc", [4, W_ADJ], f32,
                              kind="ExternalOutput")

    with tile.TileContext(nc) as tc:
        with (
            tc.tile_pool(name="const", bufs=1) as const,
            tc.tile_pool(name="xp", bufs=NCHUNK) as xp,
            tc.tile_pool(name="scratch", bufs=1) as scratch,
            tc.tile_pool(name="wp", bufs=2) as wp,
            tc.tile_pool(name="small", bufs=1) as small,
            tc.tile_pool(name="psum", bufs=1, space="PSUM") as psum,
            tc.tile_pool(name="dram", bufs=1, space="DRAM") as dram,
        ):
            # ---- constant tiles ----
            # Every adj row chunk is written as two spans with a FIXED
            # split at column L: [0, L) carries the sliding 0.5*I diagonal
            # block, [L, 8196) is identical for all chunks and carries the
            # center-link value at dest column 8192. Both DMAs are >=1 MB
            # with >=8 KB partition lines.
            L = 2048
            # zid_t: zeros with the id block at cols [L-128, L). Chunk j
            # reads the L-wide window at offset L-128*(j+1), landing the
            # id block at dest cols [j*128, (j+1)*128).
            zid_t = const.tile([P, 2 * L - P], f32)
            half_t = const.tile([P, P], f32)
            nc.gpsimd.memset(zid_t[:, 0:L - P], 0.0)
            nc.gpsimd.memset(half_t[:], C_DIAG)
            nc.gpsimd.affine_select(
                zid_t[:, L - P:L], half_t[:],
                pattern=[[1, P]],
                compare_op=mybir.AluOpType.is_equal, fill=0.0,
                base=0, channel_multiplier=-1,
            )
            nc.gpsimd.memset(zid_t[:, L:2 * L - P], 0.0)

            # zc_t: zeros with C_LINK at column N-L (dest col 8192).
            zc_t = const.tile([P, W_ADJ - L], f32)
            nc.vector.memset(zc_t[:], 0.0)
            nc.vector.memset(zc_t[:, N - L:N - L + 1], C_LINK)

            # center adj row sources (identical on every core, true layout)
            crow_t = const.tile([1, SEG], f32)
            nc.gpsimd.memset(crow_t[:], C_LINK)
            cc4_t = const.tile([4, 4], f32)
            nc.gpsimd.memset(cc4_t[:], C_CC)

            # attention weight/bias broadcast across partitions
            attw_stage = const.tile([1, C], f32)
            attw_rep = const.tile([P, C], f32)
            nc.sync.dma_start(attw_stage[:], att_w[:])
            nc.gpsimd.partition_broadcast(attw_rep[:], attw_stage[:])
            attb_stage = const.tile([1, 1], f32)
            attb_rep = const.tile([P, 1], f32)
            nc.sync.dma_start(attb_stage[:], att_b[:])
            nc.gpsimd.partition_broadcast(attb_rep[:], attb_stage[:])

            # ---- x stream: pass-through + attention partials ----
            # All 8 loads are emitted first and ride the sync (SP) HWDGE
            # ring ahead of everything else, so they drain at full HBM
            # rate; the adj bulk writes are queued on the same ring LAST,
            # giving the compute stream strict FIFO priority.
            ps = [psum.tile([1, 512], f32, name=f"ps{n}") for n in range(4)]
            ps_sum = psum.tile([1, 1], f32)

            xts = []
            for i in range(NCHUNK):
                rows = slice(i * P, (i + 1) * P)
                xt = xp.tile([P, C + 1], f32, name=f"xt{i}", tag="xt")
                nc.sync.dma_start(xt[:, 0:C], x_in[rows, :])
                nc.vector.memset(xt[:, C:C + 1], 1.0)
                xts.append(xt)

            for i in range(NCHUNK):
                rows = slice(i * P, (i + 1) * P)
                xt = xts[i]
                nc.sync.dma_start(out_x[rows, :], xt[:, 0:C])

                # wcol = sum(x * att_w, axis=1) + att_b
                # (tensor_tensor_reduce would fuse this, but it faults the
                # device on TRN2 hardware — use mul + reduce + add)
                prod = scratch.tile([P, C], f32)
                wcol = wp.tile([P, 1], f32)
                wraw = wp.tile([P, 1], f32)
                nc.vector.tensor_mul(prod[:], xt[:, 0:C], attw_rep[:])
                nc.vector.tensor_reduce(wraw[:], prod[:],
                                        axis=mybir.AxisListType.X,
                                        op=mybir.AluOpType.add)
                nc.vector.tensor_scalar_add(wcol[:], wraw[:], attb_rep[:])
                # partials += wcol.T @ [x | 1]  (accumulated over chunks)
                for n in range(4):
                    nc.tensor.matmul(ps[n][:], wcol[:],
                                     xt[:, n * 512:(n + 1) * 512],
                                     start=(i == 0), stop=(i == NCHUNK - 1))
                nc.tensor.matmul(ps_sum[:], wcol[:], xt[:, C:C + 1],
                                 start=(i == 0), stop=(i == NCHUNK - 1))

            # ---- share partials, finish centers ----
            # pad the per-rank collective buffer to 2056 f32 (8224 B) so
            # each rank's block stays 32-byte aligned
            CP = C + 8
            partial = small.tile([1, CP], f32)
            nc.vector.memset(partial[:, C:CP], 0.0)
            for n in range(4):
                nc.vector.tensor_copy(partial[:, n * 512:(n + 1) * 512],
                                      ps[n][:])
            nc.vector.tensor_copy(partial[:, C:C + 1], ps_sum[:])

            agin = dram.tile([1, CP], f32)
            agout = dram.tile([N_CORES, CP], f32)
            nc.gpsimd.dma_start(agin[:], partial[:])
            nc.gpsimd.collective_compute(
                "AllGather", mybir.AluOpType.bypass,
                replica_groups=[list(range(N_CORES))],
                ins=[agin.opt()], outs=[agout.opt()],
            )
            # segment i partial = core 2i + core 2i+1
            # Tail DMAs ride gpsimd (SWDGE) so they don't queue behind the
            # adj bulk writes on the sync ring.
            ag3 = agout[:].rearrange("(a b) c -> a b c", b=2)
            ev = small.tile([4, C + 1], f32)
            od = small.tile([4, C + 1], f32)
            nc.gpsimd.dma_start(ev[:], ag3[:, 0, 0:C + 1])
            nc.gpsimd.dma_start(od[:], ag3[:, 1, 0:C + 1])
            sums = small.tile([4, C + 1], f32)
            nc.vector.tensor_add(sums[:], ev[:], od[:])
            recip = small.tile([4, 1], f32)
            nc.vector.reciprocal(recip[:], sums[:, C:C + 1])
            cent = small.tile([4, C], f32)
            nc.vector.tensor_scalar_mul(cent[:], sums[:, 0:C], recip[:])
            nc.gpsimd.dma_start(out_centers[:], cent[:])

            # center adj rows: zero spans + c_link segment + 4x4 cc block
            # (small; on the scalar ring so they run early and free)
            for i in range(4):
                row = slice(i, i + 1)
                if i > 0:
                    nc.scalar.dma_start(out_adjc[row, 0:i * SEG],
                                        zc_t[0:1, 0:i * SEG])
                nc.scalar.dma_start(out_adjc[row, i * SEG:(i + 1) * SEG],
                                    crow_t[:])
                if i < 3:
                    nc.scalar.dma_start(out_adjc[row, (i + 1) * SEG:N],
                                        zc_t[0:1, 0:N - (i + 1) * SEG])
            nc.scalar.dma_start(out_adjc[0:4, N:N + 4], cc4_t[:])

            # ---- adj row block: 2 DMAs per 128-row chunk ----
            # Emitted last on the sync ring: strict FIFO behind the x
            # stream, so they fill all remaining HBM bandwidth.
            for j in range(NCHUNK):
                rows = slice(j * P, (j + 1) * P)
                o = L - P * (j + 1)               # sliding id window
                nc.sync.dma_start(out_adj[rows, 0:L], zid_t[:, o:o + L])
                nc.sync.dma_start(out_adj[rows, L:W_ADJ], zc_t[:])

    nc.compile()
    return nc


def _get_nc():
    global _CACHED_NC
    if _CACHED_NC is None:
        _CACHED_NC = _build()
    return _CACHED_NC


def kernel(x, att_w, att_b):
    x = np.ascontiguousarray(np.asarray(x, dtype=np.float32))
    att_w = np.ascontiguousarray(np.asarray(att_w, dtype=np.float32))
    att_b = np.asarray(att_b, dtype=np.float32).reshape(1, 1)
    assert x.shape == (N, C) and att_w.shape == (1, C)

    nc = _get_nc()
    in_maps = [
        {
            "x": np.ascontiguousarray(x[k * ROWS:(k + 1) * ROWS]),
            "att_w": att_w,
            "att_b": att_b,
        }
        for k in range(N_CORES)
    ]
    try:
        res = bass_utils.run_bass_kernel_spmd(
            nc, in_maps, core_ids=list(range(N_CORES))
        )
    except Exception:
        # one retry to ride out transient runtime/worker hiccups
        res = bass_utils.run_bass_kernel_spmd(
            nc, in_maps, core_ids=list(range(N_CORES))
        )
    return assemble(res.results)


def assemble(results):
    x_new = np.empty((N + 4, C), dtype=np.float32)
    adj = np.empty((N + 4, W_ADJ), dtype=np.float32)
    for k in range(N_CORES):
        rows = slice(k * ROWS, (k + 1) * ROWS)
        x_new[rows] = results[k]["out_x"]
        blk = results[k]["out_adj"]
        # un-rotate the core-local column layout back to global columns
        adj[rows, 0:N] = np.roll(blk[:, 0:N], k * ROWS, axis=1)
        seg = k // 2
        adj[rows, N + seg] = blk[:, N]
        rest = [N + j for j in range(4) if j != seg]
        adj[rows, rest] = blk[:, N + 1:N + 4]
    x_new[N:] = results[0]["out_centers"]
    adj[N:] = results[0]["out_adjc"]
    return x_new, adj
